# revision 32
# baseline (speedup 1.0000x reference)
"""GraphSAGE 2-layer forward on 8 Trainium2 NeuronCores — v2.

Measured bottleneck of v1: per-edge gather descriptors are generated in
software on the GpSimd Q7 cores at ~7.6 ns/descriptor, serializing 3.2 ms
of descgen (90% of runtime).  v2 removes the layer-1 gather entirely and
keeps everything else off the Q7:

  - Layer 1 messages m1 = x[src] are expanded on the HOST (pure data
    movement, like the baseline's host-built one-hot S images) and
    streamed as a dense [P, NB, D] bf16 image via HWDGE — zero
    descriptors, sequential DRAM reads at line rate.
  - Layer 2 still gathers h[src] on-device via dma_gather (h is
    device-computed), ~1.6 ms of Q7 descgen, pipelined per (sb, chunk)
    against PE/DMA consumption.
  - 1/deg is folded into the host-built S images (S entries are
    recip[dst] instead of 1.0), removing the rbc stream and the DVE
    multiply; PSUM->SBUF moves run on the Scalar engine.
  - All matmul operands are bf16 (4x PE throughput vs f32); PSUM stays
    f32.  The transposed hidden state hT is produced by weight-stationary
    batched matmuls over each super-batch (2 LDWEIGHTS per sb instead of
    2 per window).
  - The AllGather's strict all-engine barrier is dropped; layer-2's
    first h_full reader waits on the collective via Tile's AP deps.
"""

import math
import numpy as np
import ml_dtypes

import concourse.bass as bass
import concourse.bacc as bacc
import concourse.mybir as mybir
import concourse.tile as tile
from concourse.bass_utils import run_bass_kernel_spmd

P = 128          # window width == psum partitions
D = 128          # feature dim
NCORES = 8
SBW = 4          # windows per super-batch
GRP_SBS = 5      # super-batches per exchange group (also the L2 chunking)

F32 = mybir.dt.float32
BF16 = mybir.dt.bfloat16
I16 = mybir.dt.int16

BF = ml_dtypes.bfloat16


# --------------------------------------------------------------------------
# host-side planning
# --------------------------------------------------------------------------

def _build_visits(edge_s, edge_b, edge_wi, nsb, nwin_of):
    """Union-over-cores visit structure: per sb an ordered list of
    (block, window-in-sb, is_first, is_last) and a per-edge visit id."""
    presence = set(zip(edge_s.tolist(), edge_b.tolist(), edge_wi.tolist()))
    visits, vmaps = [], []
    for s in range(nsb):
        per_w = [[] for _ in range(nwin_of(s))]
        for (ss, b, wi) in presence:
            if ss == s:
                per_w[wi].append(b)
        vs, vm = [], {}
        for wi in range(nwin_of(s)):
            blocks = sorted(set(per_w[wi]))
            if not blocks:
                blocks = [0]
            for t, b in enumerate(blocks):
                vm[(b, wi)] = len(vs)
                vs.append((int(b), wi, t == 0, t == len(blocks) - 1))
        visits.append(vs)
        vmaps.append(vm)
    v_of = np.empty(edge_s.shape[0], np.int64)
    for s in range(nsb):
        vm = vmaps[s]
        if not vm:
            continue
        keys = np.array([b * SBW + wi for (b, wi) in vm.keys()], np.int64)
        vals = np.array(list(vm.values()), np.int64)
        lut = np.full(int(keys.max()) + 1, -1, np.int64)
        lut[keys] = vals
        m = edge_s == s
        v_of[m] = lut[edge_b[m] * SBW + edge_wi[m]]
    assert (v_of >= 0).all()
    return visits, v_of


def make_plan(edge_index, n_nodes, n_cores=NCORES):
    src = np.asarray(edge_index[0], dtype=np.int64)
    dst = np.asarray(edge_index[1], dtype=np.int64)
    E = src.shape[0]

    deg = np.bincount(dst, minlength=n_nodes)

    NW = int(math.ceil(n_nodes / (n_cores * P)))   # windows per core
    TOTW = NW * n_cores
    NPC = NW * P                                   # padded nodes per core
    GTOT = NPC * n_cores
    NSB = int(math.ceil(NW / SBW))
    sb_windows = [list(range(s * SBW, min((s + 1) * SBW, NW)))
                  for s in range(NSB)]
    nwin_of = lambda s: len(sb_windows[s])

    # exchange groups: contiguous runs of GRP_SBS super-batches; the h
    # exchange is one sliced AllGather per group, and layer-2's gather
    # table chunks are group-aligned so each chunk's gathers wait only
    # on that group's collective
    # small first group so the first collective (and with it the first
    # early layer-2 gather's descgen) can start as soon as possible
    sizes, rem = [], NSB
    sizes.append(min(2, rem)); rem -= sizes[-1]
    while rem > 0:
        sizes.append(min(6, rem)); rem -= sizes[-1]
    NGRP = len(sizes)
    grp_sbs, pos = [], 0
    for sz in sizes:
        grp_sbs.append(list(range(pos, pos + sz)))
        pos += sz
    sb2g = np.zeros(NSB, np.int64)
    for g, sbs in enumerate(grp_sbs):
        sb2g[sbs] = g
    own_off = []          # per group: row offset within a core's h_own
    rpc_grp = []          # per group: rows per core
    for g in range(NGRP):
        w0 = sb_windows[grp_sbs[g][0]][0]
        wn = sb_windows[grp_sbs[g][-1]][-1] + 1
        own_off.append(w0 * P)
        rpc_grp.append((wn - w0) * P)
    assert max(rpc_grp) * n_cores <= 32768
    grp_base = np.zeros(NGRP, np.int64)
    grp_base[1:] = np.cumsum([n_cores * r for r in rpc_grp])[:-1]

    # serpentine assignment of degree-sorted nodes to windows
    order = np.argsort(-deg, kind="stable")
    pos = np.arange(n_nodes)
    rnd, j = pos // TOTW, pos % TOTW
    w = np.where(rnd % 2 == 0, j, TOTW - 1 - j)
    g_sorted = w * P + rnd
    g_of_node = np.empty(n_nodes, np.int64)
    g_of_node[order] = g_sorted

    sg = g_of_node[src]
    dg = g_of_node[dst]
    e_w = dg // P                 # global dst window (core-major)
    e_dslot = (dg % P).astype(np.int64)
    e_core = e_w // NW
    e_wl = e_w % NW               # core-local window
    e_s = e_wl // SBW
    e_wi = e_wl % SBW

    recip_g = np.zeros(GTOT, np.float32)
    recip_g[g_of_node] = (1.0 / np.maximum(deg, 1)).astype(np.float32)

    # ---------------- layer-1 slot plan (host-expanded m1, no chunks) ----
    grp1 = e_core * NSB + e_s
    ordr1 = np.lexsort((e_wl, grp1))
    n1 = np.bincount(grp1, minlength=n_cores * NSB).reshape(n_cores, NSB)
    NB1 = np.maximum(np.ceil(n1.max(axis=0) / P).astype(np.int64), 1)  # [NSB]
    ob1 = np.zeros(NSB, np.int64)
    ob1[1:] = np.cumsum(NB1)[:-1]
    NB1TOT = int(NB1.sum())

    starts1 = np.searchsorted(grp1[ordr1], np.arange(n_cores * NSB))
    rank1 = np.arange(E) - starts1[grp1[ordr1]]
    p1 = rank1 % P
    b1 = rank1 // P
    assert (b1 < NB1[e_s[ordr1]]).all()
    visits1, v1_of = _build_visits(e_s[ordr1], b1, e_wi[ordr1], NSB, nwin_of)
    NV1 = [len(v) for v in visits1]
    NV1TOT = int(sum(NV1))
    ov1 = np.zeros(NSB, np.int64)
    ov1[1:] = np.cumsum(NV1)[:-1]

    # s1 image: [core, P, NV1TOT, P] bf16 with recip folded
    s1_img = np.zeros((n_cores, P, NV1TOT, P), BF)
    c1, s1s = e_core[ordr1], e_s[ordr1]
    col1 = ov1[s1s] + v1_of
    s1_img[c1, p1, col1, e_dslot[ordr1]] = \
        recip_g[dg[ordr1]].astype(BF)

    # m1 slot map (for plan_inputs): per core arrays of (p, col, src_g)
    m1_slots = []
    for k in range(n_cores):
        m = c1 == k
        m1_slots.append((p1[m], ob1[s1s[m]] + b1[m], sg[ordr1][m]))

    # ---------------- layer-2 slot plan (device gather, chunked) ---------
    # chunk = exchange group of the SRC node; index = row within the
    # group-major h_full layout [grp][core][local]
    src_k = sg // NPC
    src_loc = sg % NPC
    src_sb = (src_loc // P) // SBW
    e_chunk = sb2g[src_sb]
    own_off_a = np.asarray(own_off, np.int64)
    rpc_a = np.asarray(rpc_grp, np.int64)
    e_idx = (src_k * rpc_a[e_chunk]
             + src_loc - own_off_a[e_chunk]).astype(np.int16)
    NCH = NGRP
    run2 = (e_core * NSB + e_s) * NCH + e_chunk
    ordr2 = np.lexsort((e_wl, run2))
    n2 = np.bincount(run2, minlength=n_cores * NSB * NCH) \
        .reshape(n_cores, NSB, NCH)
    NBC = np.maximum(np.ceil(n2.max(axis=0) / P).astype(np.int64), 1)
    ob2c = np.zeros((NSB, NCH), np.int64)
    ob2c[:, 1:] = np.cumsum(NBC, axis=1)[:, :-1]
    NB2 = NBC.sum(axis=1)                      # [NSB]
    NB2max = int(NB2.max())

    starts2 = np.searchsorted(run2[ordr2], np.arange(n_cores * NSB * NCH))
    rank2 = np.arange(E) - starts2[run2[ordr2]]
    p2 = rank2 % P
    b2 = ob2c[e_s[ordr2], e_chunk[ordr2]] + rank2 // P
    assert (b2 < NB2[e_s[ordr2]]).all()
    visits2, v2_of = _build_visits(e_s[ordr2], b2, e_wi[ordr2], NSB, nwin_of)
    NV2 = [len(v) for v in visits2]
    NV2TOT = int(sum(NV2))
    ov2 = np.zeros(NSB, np.int64)
    ov2[1:] = np.cumsum(NV2)[:-1]

    s2_img = np.zeros((n_cores, P, NV2TOT, P), BF)
    c2, s2s = e_core[ordr2], e_s[ordr2]
    col2 = ov2[s2s] + v2_of
    s2_img[c2, p2, col2, e_dslot[ordr2]] = \
        recip_g[dg[ordr2]].astype(BF)

    idx16 = np.zeros((n_cores, NSB, 16, NB2max * 8), np.int16)
    idx16[c2, s2s, p2 % 16, b2 * 8 + p2 // 16] = e_idx[ordr2]
    idx_img = np.tile(idx16, (1, 1, 8, 1))

    return dict(
        n_nodes=n_nodes, E=E, n_cores=n_cores,
        NW=NW, NPC=NPC, GTOT=GTOT, NSB=NSB, NCH=NCH,
        NGRP=NGRP, grp_sbs=grp_sbs, own_off=own_off,
        rpc_grp=rpc_grp, grp_base=grp_base,
        sb_windows=sb_windows, g_of_node=g_of_node,
        NB1=NB1, ob1=ob1, NB1TOT=NB1TOT, visits1=visits1,
        NV1=NV1, ov1=ov1, NV1TOT=NV1TOT, s1_img=s1_img, m1_slots=m1_slots,
        NBC=NBC, ob2c=ob2c, NB2=NB2, NB2max=NB2max, visits2=visits2,
        NV2=NV2, ov2=ov2, NV2TOT=NV2TOT, s2_img=s2_img, idx_img=idx_img,
        # for test.py prints
        NBmax=NB2max, NB_s=NB2,
    )


def plan_inputs(plan, x, W1_l, b1, W1_r, W2_l, b2, W2_r):
    GTOT, NPC = plan["GTOT"], plan["NPC"]
    g = plan["g_of_node"]
    xp32 = np.zeros((GTOT, D), np.float32)
    xp32[g] = np.asarray(x, np.float32)
    xbf = xp32.astype(BF)

    common = dict(
        ones1=np.ones((1, P), BF),
        w1l=np.asarray(W1_l, np.float32).astype(BF),
        w1r=np.asarray(W1_r, np.float32).astype(BF),
        w2l=np.asarray(W2_l, np.float32).astype(BF),
        w2r=np.asarray(W2_r, np.float32).astype(BF),
        b1c=np.asarray(b1, np.float32).reshape(P, 1),
        b1r=np.asarray(b1, np.float32).astype(BF).reshape(1, P),
        b2r=np.asarray(b2, np.float32).astype(BF).reshape(1, P),
    )
    in_maps = []
    for k in range(plan["n_cores"]):
        m = dict(common)
        m1 = np.zeros((P, plan["NB1TOT"], D), BF)
        pp, cc, ss = plan["m1_slots"][k]
        m1[pp, cc] = xbf[ss]
        m["m1"] = m1
        m["s1"] = plan["s1_img"][k]
        m["s2"] = plan["s2_img"][k]
        m["idx"] = plan["idx_img"][k]
        m["xT"] = np.ascontiguousarray(xbf[k * NPC:(k + 1) * NPC].T)
        in_maps.append(m)
    return in_maps


# --------------------------------------------------------------------------
# device program
# --------------------------------------------------------------------------

def build_nc(plan):
    NW, NPC, GTOT = plan["NW"], plan["NPC"], plan["GTOT"]
    NSB, NCH = plan["NSB"], plan["NCH"]
    grp_base, rpc_grp = plan["grp_base"], plan["rpc_grp"]
    own_off, grp_sbs = plan["own_off"], plan["grp_sbs"]
    grp_last_sb = {sbs[-1]: g for g, sbs in enumerate(grp_sbs)}
    n_cores = plan["n_cores"]
    NB1TOT, NV1TOT = plan["NB1TOT"], plan["NV1TOT"]
    NB2max, NV2TOT = plan["NB2max"], plan["NV2TOT"]

    nc = bacc.Bacc(None, num_devices=n_cores)

    m1_t = nc.declare_dram_parameter("m1", [P, NB1TOT, D], BF16, False)
    s1_t = nc.declare_dram_parameter("s1", [P, NV1TOT, P], BF16, False)
    s2_t = nc.declare_dram_parameter("s2", [P, NV2TOT, P], BF16, False)
    idx_t = nc.declare_dram_parameter("idx", [NSB, P, NB2max * 8], I16, False)
    xT_t = nc.declare_dram_parameter("xT", [D, NPC], BF16, False)
    w1l_t = nc.declare_dram_parameter("w1l", [D, D], BF16, False)
    w1r_t = nc.declare_dram_parameter("w1r", [D, D], BF16, False)
    w2l_t = nc.declare_dram_parameter("w2l", [D, D], BF16, False)
    w2r_t = nc.declare_dram_parameter("w2r", [D, D], BF16, False)
    b1c_t = nc.declare_dram_parameter("b1c", [P, 1], F32, False)
    b1r_t = nc.declare_dram_parameter("b1r", [1, P], BF16, False)
    b2r_t = nc.declare_dram_parameter("b2r", [1, P], BF16, False)
    ones_t = nc.declare_dram_parameter("ones1", [1, P], BF16, False)
    out_t = nc.declare_dram_parameter("out", [NPC, D], F32, True)

    h_own = nc.dram_tensor("h_own", [NPC, D], BF16)
    h_full = nc.dram_tensor("h_full", [GTOT, D], BF16, addr_space="Shared")
    warm_in = nc.dram_tensor("warm_in", [1, D], BF16)
    warm_out = nc.dram_tensor("warm_out", [n_cores, D], BF16,
                              addr_space="Shared")

    RELU = mybir.ActivationFunctionType.Relu
    COPY = mybir.ActivationFunctionType.Copy

    with tile.TileContext(nc) as tc:
        NEARLY = min(4, NSB)
        with (
            tc.tile_pool(name="const", bufs=1) as constp,
            tc.tile_pool(name="pers", bufs=1) as persp,
            tc.tile_pool(name="m", bufs=2) as mp,
            tc.tile_pool(name="em", bufs=NEARLY) as emp,
            tc.tile_pool(name="eix", bufs=NEARLY) as eixp,
            tc.tile_pool(name="meta", bufs=2) as metap,
            tc.tile_pool(name="s", bufs=2) as sp,
            tc.tile_pool(name="agg", bufs=2) as aggp,
            tc.tile_pool(name="xtw", bufs=2) as xtp,
            tc.tile_pool(name="h", bufs=4) as hp,
            tc.tile_pool(name="psA", bufs=2, space=bass.MemorySpace.PSUM) as psA,
            tc.tile_pool(name="psH", bufs=2, space=bass.MemorySpace.PSUM) as psH,
            tc.tile_pool(name="psT", bufs=2, space=bass.MemorySpace.PSUM) as psT,
        ):
            # tiny warm-up collective: absorbs the one-time cc barrier
            # (~114us) at t=0, concurrent with layer-1 compute, so the real
            # exchange later doesn't pay it
            nc.gpsimd.collective_compute(
                "AllGather", mybir.AluOpType.bypass,
                replica_groups=[list(range(n_cores))],
                ins=[warm_in[:, :]], outs=[warm_out[:, :]],
            )

            ones1 = constp.tile([1, P], BF16)
            nc.sync.dma_start(ones1[:, :], ones_t[:, :])
            wts = {}
            for nm, t in (("w1l", w1l_t), ("w1r", w1r_t),
                          ("w2l", w2l_t), ("w2r", w2r_t)):
                wt = constp.tile([D, D], BF16, tag=nm)
                nc.sync.dma_start(wt[:, :], t[:, :])
                wts[nm] = wt
            b1c = constp.tile([P, 1], F32)
            nc.sync.dma_start(b1c[:, :], b1c_t[:, :])
            b1r = constp.tile([1, P], BF16)
            nc.sync.dma_start(b1r[:, :], b1r_t[:, :])
            b2r = constp.tile([1, P], BF16)
            nc.sync.dma_start(b2r[:, :], b2r_t[:, :])

            hT_own = persp.tile([D, NPC], BF16)

            # first NEARLY super-batches of layer 2 gather early: their
            # descriptors are generated while layer 1 is still computing,
            # chunk by chunk as each group's collective lands
            early_ix, early_m2 = [], []
            for s_e in range(NEARLY):
                nb_e = int(plan["NB2"][s_e])
                exe = eixp.tile([P, NB2max * 8], I16, tag="eix")
                nc.sync.dma_start(exe[:, :], idx_t[s_e, :, :])
                early_ix.append(exe)
                em_t = emp.tile([P, nb_e, D], BF16, tag="em")
                early_m2.append(em_t)

            for layer in range(2):
                s_t = s1_t if layer == 0 else s2_t
                ov = plan["ov1"] if layer == 0 else plan["ov2"]
                nvs = plan["NV1"] if layer == 0 else plan["NV2"]
                visits = plan["visits1"] if layer == 0 else plan["visits2"]
                wl_, wr_ = (wts["w1l"], wts["w1r"]) if layer == 0 \
                    else (wts["w2l"], wts["w2r"])
                brow = b1r if layer == 0 else b2r

                for s in range(NSB):
                    ws = plan["sb_windows"][s]
                    nv_s = int(nvs[s])
                    if layer == 0:
                        nb_s = int(plan["NB1"][s])
                        o1 = int(plan["ob1"][s])
                        m = mp.tile([P, nb_s, D], BF16, tag="m")
                        nc.sync.dma_start(m[:, :, :],
                                          m1_t[:, o1:o1 + nb_s, :])
                    elif s < NEARLY:
                        m = early_m2[s]          # gathered during layer 1
                    else:
                        nb_s = int(plan["NB2"][s])
                        ix = metap.tile([P, NB2max * 8], I16, tag="ix")
                        nc.sync.dma_start(ix[:, :], idx_t[s, :, :])
                        m = mp.tile([P, nb_s, D], BF16, tag="m")
                        for c in range(NCH):
                            o = int(plan["ob2c"][s, c])
                            nb = int(plan["NBC"][s, c])
                            cb = int(grp_base[c])
                            cr = n_cores * int(rpc_grp[c])
                            nc.gpsimd.dma_gather(
                                m[:, o:o + nb, :],
                                h_full[cb:cb + cr, :],
                                ix[:, o * 8:(o + nb) * 8],
                                nb * P, nb * P, D,
                                single_packet=False,
                            )
                    sv = sp.tile([P, nv_s, P], BF16, tag="sv")
                    ovs = int(ov[s])
                    nc.sync.dma_start(sv[:, :, :],
                                      s_t[:, ovs:ovs + nv_s, :])

                    aggT_ps = psA.tile([P, len(ws) * P], F32, tag="aggT_ps")
                    for v, (b, wi, st, sp_) in enumerate(visits[s]):
                        nc.tensor.matmul(
                            aggT_ps[:, wi * P:(wi + 1) * P],
                            m[:, b, :], sv[:, v, :], start=st, stop=sp_)

                    aggT = aggp.tile([P, len(ws) * P], BF16, tag="aggT")
                    nc.scalar.activation(aggT[:, :], aggT_ps[:, :], COPY)

                    if layer == 0:
                        xw = xtp.tile([P, len(ws) * P], BF16, tag="xw")
                        nc.sync.dma_start(
                            xw[:, :],
                            xT_t[:, ws[0] * P:(ws[-1] + 1) * P])

                    for wi, wl in enumerate(ws):
                        sl = slice(wi * P, (wi + 1) * P)
                        gsl = slice(wl * P, (wl + 1) * P)
                        selfT = xw[:, sl] if layer == 0 else hT_own[:, gsl]
                        hps = psH.tile([P, P], F32, tag="hps")
                        nc.tensor.matmul(hps[:, :], aggT[:, sl], wl_[:, :],
                                         start=True, stop=False)
                        nc.tensor.matmul(hps[:, :], selfT, wr_[:, :],
                                         start=False, stop=False)
                        nc.tensor.matmul(hps[:, :], ones1[:, :], brow[:, :],
                                         start=False, stop=True)
                        if layer == 0:
                            hw = hp.tile([P, P], BF16, tag="hw")
                            nc.scalar.activation(hw[:, :], hps[:, :], RELU)
                            nc.sync.dma_start(h_own[gsl, :], hw[:, :])
                        else:
                            ow = hp.tile([P, P], F32, tag="ow")
                            nc.scalar.activation(ow[:, :], hps[:, :], COPY)
                            nc.sync.dma_start(out_t[gsl, :], ow[:, :])

                    if layer == 0:
                        # transposed hidden for the layer-2 self term:
                        # hT = w1l^T aggT + w1r^T xT (weight-stationary,
                        # whole super-batch per matmul), bias+relu on ACT
                        gspan = slice(ws[0] * P, (ws[-1] + 1) * P)
                        hT_ps = psT.tile([P, len(ws) * P], F32, tag="hT_ps")
                        nc.tensor.matmul(hT_ps[:, :], wl_[:, :], aggT[:, :],
                                         start=True, stop=False)
                        nc.tensor.matmul(hT_ps[:, :], wr_[:, :], xw[:, :],
                                         start=False, stop=True)
                        nc.scalar.activation(hT_own[:, gspan], hT_ps[:, :],
                                             RELU, bias=b1c[:, :])

                        # sliced exchange: as soon as this group's h_own
                        # rows are written, all-gather them so layer-2
                        # chunks can fire without waiting for all of L1
                        if s in grp_last_sb:
                            g = grp_last_sb[s]
                            oo, rr = int(own_off[g]), int(rpc_grp[g])
                            gb = int(grp_base[g])
                            nc.gpsimd.collective_compute(
                                "AllGather", mybir.AluOpType.bypass,
                                replica_groups=[list(range(n_cores))],
                                ins=[h_own[oo:oo + rr, :]],
                                outs=[h_full[gb:gb + n_cores * rr, :]],
                            )
                            cr = n_cores * rr
                            for s_e in range(NEARLY):
                                o = int(plan["ob2c"][s_e, g])
                                nb = int(plan["NBC"][s_e, g])
                                nc.gpsimd.dma_gather(
                                    early_m2[s_e][:, o:o + nb, :],
                                    h_full[gb:gb + cr, :],
                                    early_ix[s_e][:, o * 8:(o + nb) * 8],
                                    nb * P, nb * P, D,
                                    single_packet=False,
                                )



    nc.compile()
    return nc


# --------------------------------------------------------------------------
# runner
# --------------------------------------------------------------------------

def run_plan(plan, in_maps, trace=False, **build_kw):
    nc = build_nc(plan, **build_kw)
    res = run_bass_kernel_spmd(
        nc, in_maps, list(range(plan["n_cores"])), trace=trace)
    outs = [res.results[k]["out"] for k in range(plan["n_cores"])]
    full = np.concatenate(outs, axis=0)          # [GTOT, D] in g-order
    return full[plan["g_of_node"]], res


def kernel(x, edge_index, W1_l, b1, W1_r, W2_l, b2, W2_r):
    x = np.asarray(x)
    n_nodes = x.shape[0]
    plan = make_plan(np.asarray(edge_index), n_nodes)
    in_maps = plan_inputs(plan, x, W1_l, b1, W1_r, W2_l, b2, W2_r)
    out, _ = run_plan(plan, in_maps)
    return out.astype(np.float32)


# revision 34
# speedup vs baseline: 1.0015x; 1.0015x over previous
"""GraphSAGE 2-layer forward on 8 Trainium2 NeuronCores — v2.

Measured bottleneck of v1: per-edge gather descriptors are generated in
software on the GpSimd Q7 cores at ~7.6 ns/descriptor, serializing 3.2 ms
of descgen (90% of runtime).  v2 removes the layer-1 gather entirely and
keeps everything else off the Q7:

  - Layer 1 messages m1 = x[src] are expanded on the HOST (pure data
    movement, like the baseline's host-built one-hot S images) and
    streamed as a dense [P, NB, D] bf16 image via HWDGE — zero
    descriptors, sequential DRAM reads at line rate.
  - Layer 2 still gathers h[src] on-device via dma_gather (h is
    device-computed), ~1.6 ms of Q7 descgen, pipelined per (sb, chunk)
    against PE/DMA consumption.
  - 1/deg is folded into the host-built S images (S entries are
    recip[dst] instead of 1.0), removing the rbc stream and the DVE
    multiply; PSUM->SBUF moves run on the Scalar engine.
  - All matmul operands are bf16 (4x PE throughput vs f32); PSUM stays
    f32.  The transposed hidden state hT is produced by weight-stationary
    batched matmuls over each super-batch (2 LDWEIGHTS per sb instead of
    2 per window).
  - The AllGather's strict all-engine barrier is dropped; layer-2's
    first h_full reader waits on the collective via Tile's AP deps.
"""

import math
import numpy as np
import ml_dtypes

import concourse.bass as bass
import concourse.bacc as bacc
import concourse.mybir as mybir
import concourse.tile as tile
from concourse.bass_utils import run_bass_kernel_spmd

P = 128          # window width == psum partitions
D = 128          # feature dim
NCORES = 8
SBW = 4          # windows per super-batch
GRP_SBS = 5      # super-batches per exchange group (also the L2 chunking)

F32 = mybir.dt.float32
BF16 = mybir.dt.bfloat16
I16 = mybir.dt.int16

BF = ml_dtypes.bfloat16


# --------------------------------------------------------------------------
# host-side planning
# --------------------------------------------------------------------------

def _build_visits(edge_s, edge_b, edge_wi, nsb, nwin_of):
    """Union-over-cores visit structure: per sb an ordered list of
    (block, window-in-sb, is_first, is_last) and a per-edge visit id."""
    presence = set(zip(edge_s.tolist(), edge_b.tolist(), edge_wi.tolist()))
    visits, vmaps = [], []
    for s in range(nsb):
        per_w = [[] for _ in range(nwin_of(s))]
        for (ss, b, wi) in presence:
            if ss == s:
                per_w[wi].append(b)
        vs, vm = [], {}
        for wi in range(nwin_of(s)):
            blocks = sorted(set(per_w[wi]))
            if not blocks:
                blocks = [0]
            for t, b in enumerate(blocks):
                vm[(b, wi)] = len(vs)
                vs.append((int(b), wi, t == 0, t == len(blocks) - 1))
        visits.append(vs)
        vmaps.append(vm)
    v_of = np.empty(edge_s.shape[0], np.int64)
    for s in range(nsb):
        vm = vmaps[s]
        if not vm:
            continue
        keys = np.array([b * SBW + wi for (b, wi) in vm.keys()], np.int64)
        vals = np.array(list(vm.values()), np.int64)
        lut = np.full(int(keys.max()) + 1, -1, np.int64)
        lut[keys] = vals
        m = edge_s == s
        v_of[m] = lut[edge_b[m] * SBW + edge_wi[m]]
    assert (v_of >= 0).all()
    return visits, v_of


def make_plan(edge_index, n_nodes, n_cores=NCORES):
    src = np.asarray(edge_index[0], dtype=np.int64)
    dst = np.asarray(edge_index[1], dtype=np.int64)
    E = src.shape[0]

    deg = np.bincount(dst, minlength=n_nodes)

    NW = int(math.ceil(n_nodes / (n_cores * P)))   # windows per core
    TOTW = NW * n_cores
    NPC = NW * P                                   # padded nodes per core
    GTOT = NPC * n_cores
    NSB = int(math.ceil(NW / SBW))
    sb_windows = [list(range(s * SBW, min((s + 1) * SBW, NW)))
                  for s in range(NSB)]
    nwin_of = lambda s: len(sb_windows[s])

    # exchange groups: contiguous runs of GRP_SBS super-batches; the h
    # exchange is one sliced AllGather per group, and layer-2's gather
    # table chunks are group-aligned so each chunk's gathers wait only
    # on that group's collective
    # small first group so the first collective (and with it the first
    # early layer-2 gather's descgen) can start as soon as possible
    sizes, rem = [], NSB
    sizes.append(min(2, rem)); rem -= sizes[-1]
    while rem > 0:
        sizes.append(min(6, rem)); rem -= sizes[-1]
    NGRP = len(sizes)
    grp_sbs, pos = [], 0
    for sz in sizes:
        grp_sbs.append(list(range(pos, pos + sz)))
        pos += sz
    sb2g = np.zeros(NSB, np.int64)
    for g, sbs in enumerate(grp_sbs):
        sb2g[sbs] = g
    own_off = []          # per group: row offset within a core's h_own
    rpc_grp = []          # per group: rows per core
    for g in range(NGRP):
        w0 = sb_windows[grp_sbs[g][0]][0]
        wn = sb_windows[grp_sbs[g][-1]][-1] + 1
        own_off.append(w0 * P)
        rpc_grp.append((wn - w0) * P)
    assert max(rpc_grp) * n_cores <= 32768
    grp_base = np.zeros(NGRP, np.int64)
    grp_base[1:] = np.cumsum([n_cores * r for r in rpc_grp])[:-1]

    # serpentine assignment of degree-sorted nodes to windows
    order = np.argsort(-deg, kind="stable")
    pos = np.arange(n_nodes)
    rnd, j = pos // TOTW, pos % TOTW
    w = np.where(rnd % 2 == 0, j, TOTW - 1 - j)
    g_sorted = w * P + rnd
    g_of_node = np.empty(n_nodes, np.int64)
    g_of_node[order] = g_sorted

    sg = g_of_node[src]
    dg = g_of_node[dst]
    e_w = dg // P                 # global dst window (core-major)
    e_dslot = (dg % P).astype(np.int64)
    e_core = e_w // NW
    e_wl = e_w % NW               # core-local window
    e_s = e_wl // SBW
    e_wi = e_wl % SBW

    recip_g = np.zeros(GTOT, np.float32)
    recip_g[g_of_node] = (1.0 / np.maximum(deg, 1)).astype(np.float32)

    # ---------------- layer-1 slot plan (host-expanded m1, no chunks) ----
    grp1 = e_core * NSB + e_s
    ordr1 = np.lexsort((e_wl, grp1))
    n1 = np.bincount(grp1, minlength=n_cores * NSB).reshape(n_cores, NSB)
    NB1 = np.maximum(np.ceil(n1.max(axis=0) / P).astype(np.int64), 1)  # [NSB]
    ob1 = np.zeros(NSB, np.int64)
    ob1[1:] = np.cumsum(NB1)[:-1]
    NB1TOT = int(NB1.sum())

    starts1 = np.searchsorted(grp1[ordr1], np.arange(n_cores * NSB))
    rank1 = np.arange(E) - starts1[grp1[ordr1]]
    p1 = rank1 % P
    b1 = rank1 // P
    assert (b1 < NB1[e_s[ordr1]]).all()
    visits1, v1_of = _build_visits(e_s[ordr1], b1, e_wi[ordr1], NSB, nwin_of)
    NV1 = [len(v) for v in visits1]
    NV1TOT = int(sum(NV1))
    ov1 = np.zeros(NSB, np.int64)
    ov1[1:] = np.cumsum(NV1)[:-1]

    # s1 image: [core, P, NV1TOT, P] bf16 with recip folded
    s1_img = np.zeros((n_cores, P, NV1TOT, P), BF)
    c1, s1s = e_core[ordr1], e_s[ordr1]
    col1 = ov1[s1s] + v1_of
    s1_img[c1, p1, col1, e_dslot[ordr1]] = \
        recip_g[dg[ordr1]].astype(BF)

    # m1 slot map (for plan_inputs): per core arrays of (p, col, src_g)
    m1_slots = []
    for k in range(n_cores):
        m = c1 == k
        m1_slots.append((p1[m], ob1[s1s[m]] + b1[m], sg[ordr1][m]))

    # ---------------- layer-2 slot plan (device gather, chunked) ---------
    # chunk = exchange group of the SRC node; index = row within the
    # group-major h_full layout [grp][core][local]
    src_k = sg // NPC
    src_loc = sg % NPC
    src_sb = (src_loc // P) // SBW
    e_chunk = sb2g[src_sb]
    own_off_a = np.asarray(own_off, np.int64)
    rpc_a = np.asarray(rpc_grp, np.int64)
    e_idx = (src_k * rpc_a[e_chunk]
             + src_loc - own_off_a[e_chunk]).astype(np.int16)
    NCH = NGRP
    run2 = (e_core * NSB + e_s) * NCH + e_chunk
    ordr2 = np.lexsort((e_wl, run2))
    n2 = np.bincount(run2, minlength=n_cores * NSB * NCH) \
        .reshape(n_cores, NSB, NCH)
    NBC = np.maximum(np.ceil(n2.max(axis=0) / P).astype(np.int64), 1)
    ob2c = np.zeros((NSB, NCH), np.int64)
    ob2c[:, 1:] = np.cumsum(NBC, axis=1)[:, :-1]
    NB2 = NBC.sum(axis=1)                      # [NSB]
    NB2max = int(NB2.max())

    starts2 = np.searchsorted(run2[ordr2], np.arange(n_cores * NSB * NCH))
    rank2 = np.arange(E) - starts2[run2[ordr2]]
    p2 = rank2 % P
    b2 = ob2c[e_s[ordr2], e_chunk[ordr2]] + rank2 // P
    assert (b2 < NB2[e_s[ordr2]]).all()
    visits2, v2_of = _build_visits(e_s[ordr2], b2, e_wi[ordr2], NSB, nwin_of)
    NV2 = [len(v) for v in visits2]
    NV2TOT = int(sum(NV2))
    ov2 = np.zeros(NSB, np.int64)
    ov2[1:] = np.cumsum(NV2)[:-1]

    s2_img = np.zeros((n_cores, P, NV2TOT, P), BF)
    c2, s2s = e_core[ordr2], e_s[ordr2]
    col2 = ov2[s2s] + v2_of
    s2_img[c2, p2, col2, e_dslot[ordr2]] = \
        recip_g[dg[ordr2]].astype(BF)

    # NOTE: pad slots keep index 0 (gather a real row).  Setting them to -1
    # so the Q7 trims them sounds attractive (~8% less descgen) but hangs
    # the device: the DMA-completion semaphore target is baked for the
    # untrimmed descriptor count, so trimmed gathers never complete.
    idx16 = np.zeros((n_cores, NSB, 16, NB2max * 8), np.int16)
    idx16[c2, s2s, p2 % 16, b2 * 8 + p2 // 16] = e_idx[ordr2]
    idx_img = np.tile(idx16, (1, 1, 8, 1))

    return dict(
        n_nodes=n_nodes, E=E, n_cores=n_cores,
        NW=NW, NPC=NPC, GTOT=GTOT, NSB=NSB, NCH=NCH,
        NGRP=NGRP, grp_sbs=grp_sbs, own_off=own_off,
        rpc_grp=rpc_grp, grp_base=grp_base,
        sb_windows=sb_windows, g_of_node=g_of_node,
        NB1=NB1, ob1=ob1, NB1TOT=NB1TOT, visits1=visits1,
        NV1=NV1, ov1=ov1, NV1TOT=NV1TOT, s1_img=s1_img, m1_slots=m1_slots,
        NBC=NBC, ob2c=ob2c, NB2=NB2, NB2max=NB2max, visits2=visits2,
        NV2=NV2, ov2=ov2, NV2TOT=NV2TOT, s2_img=s2_img, idx_img=idx_img,
        # for test.py prints
        NBmax=NB2max, NB_s=NB2,
    )


def plan_inputs(plan, x, W1_l, b1, W1_r, W2_l, b2, W2_r):
    GTOT, NPC = plan["GTOT"], plan["NPC"]
    g = plan["g_of_node"]
    xp32 = np.zeros((GTOT, D), np.float32)
    xp32[g] = np.asarray(x, np.float32)
    xbf = xp32.astype(BF)

    common = dict(
        ones1=np.ones((1, P), BF),
        w1l=np.asarray(W1_l, np.float32).astype(BF),
        w1r=np.asarray(W1_r, np.float32).astype(BF),
        w2l=np.asarray(W2_l, np.float32).astype(BF),
        w2r=np.asarray(W2_r, np.float32).astype(BF),
        b1c=np.asarray(b1, np.float32).reshape(P, 1),
        b1r=np.asarray(b1, np.float32).astype(BF).reshape(1, P),
        b2r=np.asarray(b2, np.float32).astype(BF).reshape(1, P),
    )
    in_maps = []
    for k in range(plan["n_cores"]):
        m = dict(common)
        m1 = np.zeros((P, plan["NB1TOT"], D), BF)
        pp, cc, ss = plan["m1_slots"][k]
        m1[pp, cc] = xbf[ss]
        m["m1"] = m1
        m["s1"] = plan["s1_img"][k]
        m["s2"] = plan["s2_img"][k]
        m["idx"] = plan["idx_img"][k]
        m["xT"] = np.ascontiguousarray(xbf[k * NPC:(k + 1) * NPC].T)
        in_maps.append(m)
    return in_maps


# --------------------------------------------------------------------------
# device program
# --------------------------------------------------------------------------

def build_nc(plan):
    NW, NPC, GTOT = plan["NW"], plan["NPC"], plan["GTOT"]
    NSB, NCH = plan["NSB"], plan["NCH"]
    grp_base, rpc_grp = plan["grp_base"], plan["rpc_grp"]
    own_off, grp_sbs = plan["own_off"], plan["grp_sbs"]
    grp_last_sb = {sbs[-1]: g for g, sbs in enumerate(grp_sbs)}
    n_cores = plan["n_cores"]
    NB1TOT, NV1TOT = plan["NB1TOT"], plan["NV1TOT"]
    NB2max, NV2TOT = plan["NB2max"], plan["NV2TOT"]

    nc = bacc.Bacc(None, num_devices=n_cores)

    m1_t = nc.declare_dram_parameter("m1", [P, NB1TOT, D], BF16, False)
    s1_t = nc.declare_dram_parameter("s1", [P, NV1TOT, P], BF16, False)
    s2_t = nc.declare_dram_parameter("s2", [P, NV2TOT, P], BF16, False)
    idx_t = nc.declare_dram_parameter("idx", [NSB, P, NB2max * 8], I16, False)
    xT_t = nc.declare_dram_parameter("xT", [D, NPC], BF16, False)
    w1l_t = nc.declare_dram_parameter("w1l", [D, D], BF16, False)
    w1r_t = nc.declare_dram_parameter("w1r", [D, D], BF16, False)
    w2l_t = nc.declare_dram_parameter("w2l", [D, D], BF16, False)
    w2r_t = nc.declare_dram_parameter("w2r", [D, D], BF16, False)
    b1c_t = nc.declare_dram_parameter("b1c", [P, 1], F32, False)
    b1r_t = nc.declare_dram_parameter("b1r", [1, P], BF16, False)
    b2r_t = nc.declare_dram_parameter("b2r", [1, P], BF16, False)
    ones_t = nc.declare_dram_parameter("ones1", [1, P], BF16, False)
    out_t = nc.declare_dram_parameter("out", [NPC, D], F32, True)

    h_own = nc.dram_tensor("h_own", [NPC, D], BF16)
    h_full = nc.dram_tensor("h_full", [GTOT, D], BF16, addr_space="Shared")
    warm_in = nc.dram_tensor("warm_in", [1, D], BF16)
    warm_out = nc.dram_tensor("warm_out", [n_cores, D], BF16,
                              addr_space="Shared")

    RELU = mybir.ActivationFunctionType.Relu
    COPY = mybir.ActivationFunctionType.Copy

    with tile.TileContext(nc) as tc:
        NEARLY = min(4, NSB)
        with (
            tc.tile_pool(name="const", bufs=1) as constp,
            tc.tile_pool(name="pers", bufs=1) as persp,
            tc.tile_pool(name="m", bufs=2) as mp,
            tc.tile_pool(name="em", bufs=NEARLY) as emp,
            tc.tile_pool(name="eix", bufs=NEARLY) as eixp,
            tc.tile_pool(name="meta", bufs=2) as metap,
            tc.tile_pool(name="s", bufs=2) as sp,
            tc.tile_pool(name="agg", bufs=2) as aggp,
            tc.tile_pool(name="xtw", bufs=2) as xtp,
            tc.tile_pool(name="h", bufs=4) as hp,
            tc.tile_pool(name="psA", bufs=2, space=bass.MemorySpace.PSUM) as psA,
            tc.tile_pool(name="psH", bufs=2, space=bass.MemorySpace.PSUM) as psH,
            tc.tile_pool(name="psT", bufs=2, space=bass.MemorySpace.PSUM) as psT,
        ):
            # tiny warm-up collective: absorbs the one-time cc barrier
            # (~114us) at t=0, concurrent with layer-1 compute, so the real
            # exchange later doesn't pay it
            nc.gpsimd.collective_compute(
                "AllGather", mybir.AluOpType.bypass,
                replica_groups=[list(range(n_cores))],
                ins=[warm_in[:, :]], outs=[warm_out[:, :]],
            )

            ones1 = constp.tile([1, P], BF16)
            nc.sync.dma_start(ones1[:, :], ones_t[:, :])
            wts = {}
            for nm, t in (("w1l", w1l_t), ("w1r", w1r_t),
                          ("w2l", w2l_t), ("w2r", w2r_t)):
                wt = constp.tile([D, D], BF16, tag=nm)
                nc.sync.dma_start(wt[:, :], t[:, :])
                wts[nm] = wt
            b1c = constp.tile([P, 1], F32)
            nc.sync.dma_start(b1c[:, :], b1c_t[:, :])
            b1r = constp.tile([1, P], BF16)
            nc.sync.dma_start(b1r[:, :], b1r_t[:, :])
            b2r = constp.tile([1, P], BF16)
            nc.sync.dma_start(b2r[:, :], b2r_t[:, :])

            hT_own = persp.tile([D, NPC], BF16)

            # first NEARLY super-batches of layer 2 gather early: their
            # descriptors are generated while layer 1 is still computing,
            # chunk by chunk as each group's collective lands
            early_ix, early_m2 = [], []
            for s_e in range(NEARLY):
                nb_e = int(plan["NB2"][s_e])
                exe = eixp.tile([P, NB2max * 8], I16, tag="eix")
                nc.sync.dma_start(exe[:, :], idx_t[s_e, :, :])
                early_ix.append(exe)
                em_t = emp.tile([P, nb_e, D], BF16, tag="em")
                early_m2.append(em_t)

            for layer in range(2):
                s_t = s1_t if layer == 0 else s2_t
                ov = plan["ov1"] if layer == 0 else plan["ov2"]
                nvs = plan["NV1"] if layer == 0 else plan["NV2"]
                visits = plan["visits1"] if layer == 0 else plan["visits2"]
                wl_, wr_ = (wts["w1l"], wts["w1r"]) if layer == 0 \
                    else (wts["w2l"], wts["w2r"])
                brow = b1r if layer == 0 else b2r

                for s in range(NSB):
                    ws = plan["sb_windows"][s]
                    nv_s = int(nvs[s])
                    if layer == 0:
                        nb_s = int(plan["NB1"][s])
                        o1 = int(plan["ob1"][s])
                        m = mp.tile([P, nb_s, D], BF16, tag="m")
                        nc.sync.dma_start(m[:, :, :],
                                          m1_t[:, o1:o1 + nb_s, :])
                    elif s < NEARLY:
                        m = early_m2[s]          # gathered during layer 1
                    else:
                        nb_s = int(plan["NB2"][s])
                        ix = metap.tile([P, NB2max * 8], I16, tag="ix")
                        nc.sync.dma_start(ix[:, :], idx_t[s, :, :])
                        m = mp.tile([P, nb_s, D], BF16, tag="m")
                        for c in range(NCH):
                            o = int(plan["ob2c"][s, c])
                            nb = int(plan["NBC"][s, c])
                            cb = int(grp_base[c])
                            cr = n_cores * int(rpc_grp[c])
                            nc.gpsimd.dma_gather(
                                m[:, o:o + nb, :],
                                h_full[cb:cb + cr, :],
                                ix[:, o * 8:(o + nb) * 8],
                                nb * P, nb * P, D,
                                single_packet=False,
                            )
                    sv = sp.tile([P, nv_s, P], BF16, tag="sv")
                    ovs = int(ov[s])
                    nc.sync.dma_start(sv[:, :, :],
                                      s_t[:, ovs:ovs + nv_s, :])

                    aggT_ps = psA.tile([P, len(ws) * P], F32, tag="aggT_ps")
                    for v, (b, wi, st, sp_) in enumerate(visits[s]):
                        nc.tensor.matmul(
                            aggT_ps[:, wi * P:(wi + 1) * P],
                            m[:, b, :], sv[:, v, :], start=st, stop=sp_)

                    aggT = aggp.tile([P, len(ws) * P], BF16, tag="aggT")
                    nc.scalar.activation(aggT[:, :], aggT_ps[:, :], COPY)

                    if layer == 0:
                        xw = xtp.tile([P, len(ws) * P], BF16, tag="xw")
                        nc.sync.dma_start(
                            xw[:, :],
                            xT_t[:, ws[0] * P:(ws[-1] + 1) * P])

                    for wi, wl in enumerate(ws):
                        sl = slice(wi * P, (wi + 1) * P)
                        gsl = slice(wl * P, (wl + 1) * P)
                        selfT = xw[:, sl] if layer == 0 else hT_own[:, gsl]
                        hps = psH.tile([P, P], F32, tag="hps")
                        nc.tensor.matmul(hps[:, :], aggT[:, sl], wl_[:, :],
                                         start=True, stop=False)
                        nc.tensor.matmul(hps[:, :], selfT, wr_[:, :],
                                         start=False, stop=False)
                        nc.tensor.matmul(hps[:, :], ones1[:, :], brow[:, :],
                                         start=False, stop=True)
                        if layer == 0:
                            hw = hp.tile([P, P], BF16, tag="hw")
                            nc.scalar.activation(hw[:, :], hps[:, :], RELU)
                            nc.sync.dma_start(h_own[gsl, :], hw[:, :])
                        else:
                            ow = hp.tile([P, P], F32, tag="ow")
                            nc.scalar.activation(ow[:, :], hps[:, :], COPY)
                            nc.sync.dma_start(out_t[gsl, :], ow[:, :])

                    if layer == 0:
                        # transposed hidden for the layer-2 self term:
                        # hT = w1l^T aggT + w1r^T xT (weight-stationary,
                        # whole super-batch per matmul), bias+relu on ACT
                        gspan = slice(ws[0] * P, (ws[-1] + 1) * P)
                        hT_ps = psT.tile([P, len(ws) * P], F32, tag="hT_ps")
                        nc.tensor.matmul(hT_ps[:, :], wl_[:, :], aggT[:, :],
                                         start=True, stop=False)
                        nc.tensor.matmul(hT_ps[:, :], wr_[:, :], xw[:, :],
                                         start=False, stop=True)
                        nc.scalar.activation(hT_own[:, gspan], hT_ps[:, :],
                                             RELU, bias=b1c[:, :])

                        # sliced exchange: as soon as this group's h_own
                        # rows are written, all-gather them so layer-2
                        # chunks can fire without waiting for all of L1
                        if s in grp_last_sb:
                            g = grp_last_sb[s]
                            oo, rr = int(own_off[g]), int(rpc_grp[g])
                            gb = int(grp_base[g])
                            nc.gpsimd.collective_compute(
                                "AllGather", mybir.AluOpType.bypass,
                                replica_groups=[list(range(n_cores))],
                                ins=[h_own[oo:oo + rr, :]],
                                outs=[h_full[gb:gb + n_cores * rr, :]],
                            )
                            cr = n_cores * rr
                            for s_e in range(NEARLY):
                                o = int(plan["ob2c"][s_e, g])
                                nb = int(plan["NBC"][s_e, g])
                                nc.gpsimd.dma_gather(
                                    early_m2[s_e][:, o:o + nb, :],
                                    h_full[gb:gb + cr, :],
                                    early_ix[s_e][:, o * 8:(o + nb) * 8],
                                    nb * P, nb * P, D,
                                    single_packet=False,
                                )



    nc.compile()
    return nc


# --------------------------------------------------------------------------
# runner
# --------------------------------------------------------------------------

def run_plan(plan, in_maps, trace=False, **build_kw):
    nc = build_nc(plan, **build_kw)
    res = run_bass_kernel_spmd(
        nc, in_maps, list(range(plan["n_cores"])), trace=trace)
    outs = [res.results[k]["out"] for k in range(plan["n_cores"])]
    full = np.concatenate(outs, axis=0)          # [GTOT, D] in g-order
    return full[plan["g_of_node"]], res


def kernel(x, edge_index, W1_l, b1, W1_r, W2_l, b2, W2_r):
    x = np.asarray(x)
    n_nodes = x.shape[0]
    plan = make_plan(np.asarray(edge_index), n_nodes)
    in_maps = plan_inputs(plan, x, W1_l, b1, W1_r, W2_l, b2, W2_r)
    out, _ = run_plan(plan, in_maps)
    return out.astype(np.float32)
